# revision 1
# baseline (speedup 1.0000x reference)
"""Trainium2 Bass kernel for CLRNet SimOTA-style assignment (B=8, N=4096, M=32, K=72).

v2: the O(N*M*K) subtract runs on the TensorEngine as a delta-pattern matmul
with exact 3-way bf16 splitting of fp32 operands (products are x1/x0/x-1 so
each term is exact; fp32 PSUM accumulation reconstructs p - t to ~1ulp).
PSUM layout is k-major (col = kk*M + m) so 512-column matmul slices align to
PSUM banks and the per-m K-reduction is a strided-view DVE reduce.

Two reduce paths per tile, mixed to balance engines:
  d: DVE tensor_reduce(abs) straight from PSUM
  m: ACT abs -> SBUF, GPSIMD max-fold via |a|+|b| = max(|a+b|,|a-b|)
     (PE emits u=a+b, v=a-b columns), then an 18-wide DVE reduce
Phase 2/3 cost assembly follows the v1 structure; conflict-resolution cm is
fused into one scalar_tensor_tensor on unmasked cost.
"""

import os
import sys

sys.path.insert(0, "/opt/trn_rl_repo")

import numpy as np

B, N, M, K = 8, 4096, 32, 72
D_FEAT = 78
IMG_W = 800.0
BIG = 100000.0
BIG2 = 100000.0
EPS = 1e-12

KH = 36          # k per half
NPAIR = KH // 2  # u/v pairs per half

_CACHE = {}

# tiles (of 32) using the ACT+GPSIMD max-fold path; rest use DVE-direct
import os as _os
M_TILES = (frozenset() if _os.environ.get('V2_NO_MPATH') else frozenset(t for t in range(32) if t % 3 != 0))


def _build_nc(n=N, reps=1):
    import concourse.bass as bass
    import concourse.bacc as bacc
    import concourse.mybir as mybir
    from concourse.tile import TileContext

    Alu = mybir.AluOpType
    ACT = mybir.ActivationFunctionType
    dt = mybir.dt
    X = mybir.AxisListType.X
    XY = mybir.AxisListType.XY

    P = 128
    T = n // P
    TM = T * M

    f32 = dt.float32
    bf16 = dt.bfloat16

    nc = bacc.Bacc()

    preds = nc.declare_dram_parameter("preds", [n, D_FEAT], f32, isOutput=False)
    targets = nc.declare_dram_parameter("targets", [M, D_FEAT], f32, isOutput=False)
    maskf = nc.declare_dram_parameter("maskf", [M], f32, isOutput=False)
    out_assigned = nc.declare_dram_parameter("out_assigned", [n], dt.int32, isOutput=True)
    out_matched = nc.declare_dram_parameter("out_matched", [n], dt.int32, isOutput=True)

    scr_td = nc.dram_tensor("scr_td", [3, K, M], bf16)      # -t2 splits (j, k, m)
    scr_v = nc.dram_tensor("scr_v", [K, M], bf16)           # validity v^T
    scr_ones = nc.dram_tensor("scr_ones", [4096], bf16)
    scr_small = nc.dram_tensor("scr_small", [8, M], f32)
    scr_mx2 = nc.dram_tensor("scr_mx2", [P, 2], f32)
    scr_g2 = nc.dram_tensor("scr_g2", [2, 1], f32)
    scr_thr = nc.dram_tensor("scr_thr", [M, 1], f32)
    scr_g = nc.dram_tensor("scr_g", [3, 1], f32)

    RHS_C = KH * M  # 1152 delta cols

    with TileContext(nc) as tc:
        with (
            tc.tile_pool(name="const", bufs=1) as cpool,
            tc.tile_pool(name="sb", bufs=1) as pool,
            tc.tile_pool(name="absd", bufs=3) as apool,
            tc.tile_pool(name="mx", bufs=3) as mpool,
        ):
            # ---------------- constants ----------------
            icol = cpool.tile([P, 1], f32)
            nc.gpsimd.iota(icol[:], pattern=[[0, 1]], channel_multiplier=1,
                           allow_small_or_imprecise_dtypes=True)
            irow = cpool.tile([P, P], f32)
            nc.gpsimd.iota(irow[:], pattern=[[1, P]], channel_multiplier=0,
                           allow_small_or_imprecise_dtypes=True)
            ident = cpool.tile([P, P], f32)
            nc.vector.tensor_scalar(ident[:], irow[:], icol[:], None, Alu.is_equal)
            identb = cpool.tile([P, P], bf16)
            nc.vector.tensor_copy(identb[:], ident[:])
            bmi = cpool.tile([P, M], f32)
            nc.gpsimd.iota(bmi[:], pattern=[[-1, M]], base=int(BIG2),
                           channel_multiplier=0, allow_small_or_imprecise_dtypes=True)
            iota4 = cpool.tile([P, 4], f32)
            nc.gpsimd.iota(iota4[:], pattern=[[1, 4]], base=1, channel_multiplier=0,
                           allow_small_or_imprecise_dtypes=True)
            epsc = cpool.tile([P, 1], f32)
            nc.gpsimd.memset(epsc[:], EPS)

            # lhsT row order (PE quadrant alignment): rows 0..31 split-a kk'0..31,
            # 32..63 split-b, 64..95 split-c, 96..99 a-tail kk'32..35,
            # 100..103 b-tail, 104..107 c-tail, 108..110 ones.
            # rowkk[r] = local kk' for row r (same for all three splits).
            icol108 = cpool.tile([108, 1], f32)
            nc.gpsimd.iota(icol108[:], pattern=[[0, 1]], channel_multiplier=1,
                           allow_small_or_imprecise_dtypes=True)
            rowkk = cpool.tile([108, 1], f32)
            rk_t = cpool.tile([108, 1], f32)
            nc.vector.tensor_copy(rowkk[:], icol108[:])
            for thrv, w in ((32.0, -32.0), (64.0, -32.0), (100.0, -4.0), (104.0, -4.0)):
                nc.vector.tensor_scalar(rk_t[:], icol108[:], thrv, w, Alu.is_ge, Alu.mult)
                nc.vector.tensor_tensor(rowkk[:], rowkk[:], rk_t[:], Alu.add)

            # delta patterns, full 108 rows; cols (kk, m) / (uv, j, m)
            cb = tc.alloc_tile_pool(name="cbuild", bufs=1)
            kkcol = cb.tile([108, KH * M], f32)
            nc.gpsimd.iota(kkcol[:].rearrange("p (kk m) -> p kk m", m=M),
                           pattern=[[1, KH], [0, M]], channel_multiplier=0,
                           allow_small_or_imprecise_dtypes=True)
            deltaf = cb.tile([108, KH * M], f32)
            nc.vector.tensor_scalar(deltaf[:], kkcol[:], rowkk[:], None, Alu.is_equal)
            deltab = cpool.tile([108, KH * M], bf16)
            nc.vector.tensor_copy(deltab[:], deltaf[:])
            onesmall = cpool.tile([P, 32], bf16)
            nc.gpsimd.memset(onesmall[:], 1.0)
            nc.sync.dma_start(out=scr_ones[:].rearrange("(p f) -> p f", p=P),
                              in_=onesmall[:])


            lhsT = [cpool.tile([111, n], bf16, tag=f"lhsT{h}", name=f"lhsT{h}") for h in range(2)]
            rhs_d = [cpool.tile([111, RHS_C], bf16, tag=f"rhsd{h}", name=f"rhsd{h}") for h in range(2)]
            for h in range(2):
                nc.sync.dma_start(
                    out=lhsT[h][108:111, :],
                    in_=scr_ones[:].unsqueeze(0).broadcast_to([3, n]))
            cb.release()

            # ---------------- body ----------------
            def _body():
                P_sb = pool.tile([P, T * D_FEAT], f32)
                pview = preds[:].rearrange("(t p) d -> p t d", p=P)
                tch = max(1, T // 4)
                for t0 in range(0, T, tch):
                    t1 = min(T, t0 + tch)
                    nc.sync.dma_start(
                        out=P_sb[:].rearrange("p (t d) -> p t d", d=D_FEAT)[:, t0:t1],
                        in_=pview[:, t0:t1],
                    )
                T_sb = pool.tile([M, D_FEAT], f32)
                nc.sync.dma_start(out=T_sb[:], in_=targets[:])
                Mk = pool.tile([M, 1], f32)
                nc.sync.dma_start(out=Mk[:], in_=maskf[:].unsqueeze(1))

                Pv = P_sb[:].rearrange("p (t d) -> p t d", d=D_FEAT)
                d3 = lambda ap: ap.rearrange("p (t m) -> p t m", m=M)

                # ------------ target-side precompute (M-partition layout) ------
                tdx = T_sb[:, 6:78]
                v0 = pool.tile([M, K], f32)
                nc.vector.tensor_scalar(v0[:], tdx, 0.0, None, Alu.is_ge)
                v1 = pool.tile([M, K], f32)
                nc.vector.tensor_scalar(v1[:], tdx, IMG_W, None, Alu.is_lt)
                vv = pool.tile([M, K], f32)
                nc.vector.tensor_tensor(vv[:], v0[:], v1[:], Alu.mult)
                nt2 = pool.tile([M, K], f32)  # -t2
                nc.vector.tensor_tensor(nt2[:], tdx, vv[:], Alu.mult)
                nc.vector.tensor_scalar(nt2[:], nt2[:], -1.0, None, Alu.mult)
                onemv = pool.tile([M, K], f32)
                nc.vector.tensor_scalar(onemv[:], vv[:], -1.0, 1.0, Alu.mult, Alu.add)
                lenr = pool.tile([M, 1], f32)
                nc.vector.tensor_reduce(lenr[:], vv[:], axis=X, op=Alu.add)
                lenc = pool.tile([M, 1], f32)
                nc.vector.tensor_scalar(lenc[:], lenr[:], 1.0, None, Alu.max)
                invlen = pool.tile([M, 1], f32)
                nc.vector.reciprocal(invlen[:], lenc[:])
                a30 = pool.tile([M, 1], f32)
                nc.vector.tensor_scalar(a30[:], lenr[:], 30.0, None, Alu.mult)
                aeps = pool.tile([M, 1], f32)
                nc.vector.tensor_scalar(aeps[:], a30[:], 1e-9, None, Alu.add)

                # -t2 bf16 triple split
                tda = pool.tile([M, K], bf16)
                nc.vector.tensor_copy(tda[:], nt2[:])
                tr1 = pool.tile([M, K], f32)
                nc.vector.tensor_tensor(tr1[:], nt2[:], tda[:], Alu.subtract)
                tdb = pool.tile([M, K], bf16)
                nc.vector.tensor_copy(tdb[:], tr1[:])
                tr2 = pool.tile([M, K], f32)
                nc.vector.tensor_tensor(tr2[:], tr1[:], tdb[:], Alu.subtract)
                tdc = pool.tile([M, K], bf16)
                nc.vector.tensor_copy(tdc[:], tr2[:])
                for j, tt in enumerate((tda, tdb, tdc)):
                    nc.sync.dma_start(out=scr_td[j].rearrange("k m -> m k"), in_=tt[:])
                for h in range(2):
                    nc.sync.dma_start(
                        out=rhs_d[h][108:111, 0:KH * M].rearrange(
                            "p (k m) -> p k m", m=M),
                        in_=scr_td[:, h * KH:(h + 1) * KH, :],
                    )
                vvb = pool.tile([M, K], bf16)
                nc.vector.tensor_copy(vvb[:], vv[:])
                nc.sync.dma_start(out=scr_v[:].rearrange("k m -> m k"), in_=vvb[:])
                for h in range(2):
                    vrow = pool.tile([108, KH * M], bf16, name=f"vrow{h}")
                    nc.sync.dma_start(
                        out=vrow[:],
                        in_=scr_v[h * KH:(h + 1) * KH, :].flatten()
                            .unsqueeze(0).broadcast_to([108, KH * M]))
                    nc.vector.tensor_tensor(
                        rhs_d[h][0:108, 0:KH * M], deltab[:], vrow[:], Alu.mult)

                # small per-m rows -> SRep broadcast
                bigoff = pool.tile([M, 1], f32)
                nc.vector.tensor_scalar(bigoff[:], Mk[:], -BIG, BIG, Alu.mult, Alu.add)
                negbig = pool.tile([M, 1], f32)
                nc.vector.tensor_scalar(negbig[:], bigoff[:], -1.0, None, Alu.mult)
                spack = pool.tile([M, 8], f32)
                nc.vector.tensor_copy(spack[:, 0:1], T_sb[:, 2:3])
                nc.vector.tensor_copy(spack[:, 1:2], T_sb[:, 3:4])
                nc.vector.tensor_copy(spack[:, 2:3], T_sb[:, 4:5])
                nc.vector.tensor_copy(spack[:, 3:4], T_sb[:, 1:2])
                nc.vector.tensor_copy(spack[:, 4:5], invlen[:])
                nc.vector.tensor_copy(spack[:, 5:6], Mk[:])
                nc.vector.tensor_copy(spack[:, 6:7], bigoff[:])
                nc.vector.tensor_copy(spack[:, 7:8], Mk[:])
                nc.sync.dma_start(out=scr_small[:].rearrange("i m -> m i"), in_=spack[:])
                SRep = pool.tile([P, 8 * M], f32)
                nc.sync.dma_start(
                    out=SRep[:],
                    in_=scr_small[:].flatten().unsqueeze(0).broadcast_to([P, 8 * M]),
                )

                def srep(i):
                    return SRep[:, i * M:(i + 1) * M].unsqueeze(1).broadcast_to([P, T, M])

                # ------------ fused split/transpose/heavy pipeline -------------
                px = Pv[:, :, 6:78]
                DrawH = pool.tile([P, T * 2 * M], f32)
                Dr3 = DrawH[:].rearrange("p (t h m) -> p t h m", h=2, m=M)
                Dm = pool.tile([P, TM], f32)

                ftp = tc.alloc_tile_pool(name="ftmp", bufs=12)

                def ft(name):
                    return ftp.tile([P, TM], f32, tag="ft", name=name)

                def pcol(c):
                    return Pv[:, :, c].unsqueeze(2).broadcast_to([P, T, M])

                # independent phase-2 branch (no Dm dependency) — emitted
                # early so it fills engine gaps during the heavy stage
                dxf = ft("dxf")
                nc.vector.tensor_tensor(d3(dxf[:]), pcol(2), srep(0), Alu.subtract)
                dyf = ft("dyf")
                nc.vector.tensor_tensor(d3(dyf[:]), pcol(3), srep(1), Alu.subtract)
                nc.scalar.activation(dxf[:], dxf[:], ACT.Square)
                nc.scalar.activation(dyf[:], dyf[:], ACT.Square)
                xyf = ft("xyf")
                nc.gpsimd.tensor_tensor(xyf[:], dxf[:], dyf[:], Alu.add)
                nc.scalar.activation(xyf[:], xyf[:], ACT.Sqrt)
                thf = ft("thf")
                nc.vector.tensor_tensor(d3(thf[:]), pcol(4), srep(2), Alu.subtract)
                nc.scalar.activation(thf[:], thf[:], ACT.Abs)

                lg = Pv[:, :, 0:2]
                sig = pool.tile([P, T * 2], f32)
                nc.scalar.activation(sig[:].rearrange("p (t c) -> p t c", c=2), lg,
                                     ACT.Sigmoid)
                qq = pool.tile([P, T * 2], f32)
                nc.vector.tensor_scalar(qq[:], sig[:], -1.0, 1.0, Alu.mult, Alu.add)
                lp = pool.tile([P, T * 2], f32)
                nc.scalar.activation(lp[:], sig[:], ACT.Ln, bias=epsc[:])
                lq = pool.tile([P, T * 2], f32)
                nc.scalar.activation(lq[:], qq[:], ACT.Ln, bias=epsc[:])
                p2 = pool.tile([P, T * 2], f32)
                nc.vector.tensor_tensor(p2[:], sig[:], sig[:], Alu.mult)
                q2 = pool.tile([P, T * 2], f32)
                nc.vector.tensor_tensor(q2[:], qq[:], qq[:], Alu.mult)
                pos = pool.tile([P, T * 2], f32)
                nc.vector.scalar_tensor_tensor(pos[:], lp[:], -0.25, q2[:],
                                               Alu.mult, Alu.mult)
                neg = pool.tile([P, T * 2], f32)
                nc.vector.scalar_tensor_tensor(neg[:], lq[:], -0.75, p2[:],
                                               Alu.mult, Alu.mult)
                fdiff = pool.tile([P, T * 2], f32)
                nc.vector.tensor_tensor(fdiff[:], pos[:], neg[:], Alu.subtract)
                fv = fdiff[:].rearrange("p (t c) -> p t c", c=2)
                d0b = fv[:, :, 0].unsqueeze(2).broadcast_to([P, T, M])
                ddt = pool.tile([P, T], f32)
                nc.vector.tensor_tensor(ddt[:], fv[:, :, 1], fv[:, :, 0], Alu.subtract)
                ddb = ddt[:].unsqueeze(2).broadcast_to([P, T, M])
                cls = ft("cls")
                nc.gpsimd.tensor_tensor(d3(cls[:]), srep(3), ddb, Alu.mult)
                nc.gpsimd.tensor_tensor(d3(cls[:]), d3(cls[:]), d0b, Alu.add)

                sprep = tc.alloc_tile_pool(name="sprep", bufs=2)
                CH = 8
                with (
                    tc.tile_pool(name="pdiff", bufs=2, space="PSUM") as pdiff,
                    tc.tile_pool(name="psA", bufs=1, space="PSUM") as psplit,
                ):
                    for c in range(T // CH):
                        ts0 = c * CH
                        sl = slice(ts0, ts0 + CH)
                        pa = sprep.tile([P, CH * K], bf16, tag="pa", name="pa")
                        pav = pa[:].rearrange("p (t k) -> p t k", k=K)
                        nc.scalar.activation(pav, px[:, sl], ACT.Copy)
                        pr1 = sprep.tile([P, CH * K], f32, tag="pr1", name="pr1")
                        pr1v = pr1[:].rearrange("p (t k) -> p t k", k=K)
                        nc.gpsimd.tensor_tensor(pr1v, px[:, sl], pav, Alu.subtract)
                        pb = sprep.tile([P, CH * K], bf16, tag="pb", name="pb")
                        pbv = pb[:].rearrange("p (t k) -> p t k", k=K)
                        nc.scalar.activation(pbv, pr1v, ACT.Copy)
                        pr2 = sprep.tile([P, CH * K], f32, tag="pr2", name="pr2")
                        pr2v = pr2[:].rearrange("p (t k) -> p t k", k=K)
                        nc.gpsimd.tensor_tensor(pr2v, pr1v, pbv, Alu.subtract)
                        pc = sprep.tile([P, CH * K], bf16, tag="pc", name="pc")
                        pcv = pc[:].rearrange("p (t k) -> p t k", k=K)
                        nc.scalar.activation(pcv, pr2v, ACT.Copy)

                        ctails = []
                        for h in range(2):
                            tl = sprep.tile([P, CH * 44], bf16, tag=f"ct{h}",
                                            name=f"ctails{h}")
                            tlv = tl[:].rearrange("p (t j) -> p t j", j=44)
                            nc.vector.tensor_copy(
                                tlv[:, :, 0:32], pcv[:, :, h * KH:h * KH + 32])
                            for j, sp in enumerate((pav, pbv, pcv)):
                                nc.vector.tensor_copy(
                                    tlv[:, :, 32 + 4 * j:36 + 4 * j],
                                    sp[:, :, h * KH + 32:h * KH + 36])
                            ctails.append(tlv)

                        psT = [psplit.tile([108, CH * P], bf16, tag=f"psT{h}",
                                           name=f"psT{h}") for h in range(2)]
                        for ti in range(CH):
                            for h in range(2):
                                for j, sp in enumerate((pav, pbv)):
                                    nc.tensor.transpose(
                                        psT[h][32 * j:32 * (j + 1),
                                               ti * P:(ti + 1) * P],
                                        sp[:, ti, h * KH:h * KH + 32],
                                        identb[:],
                                    )
                                nc.tensor.transpose(
                                    psT[h][64:108, ti * P:(ti + 1) * P],
                                    ctails[h][:, ti, :],
                                    identb[:],
                                )
                        for h in range(2):
                            nc.scalar.activation(
                                lhsT[h][0:108, ts0 * P:(ts0 + CH) * P],
                                psT[h][:], ACT.Copy)

                        for t in range(ts0, ts0 + CH):
                            use_m = t in M_TILES
                            for h in range(2):
                                lslice = lhsT[h][:, t * P:(t + 1) * P]
                                dps = pdiff.tile([P, 1536], f32, tag="dps")
                                for c0, c1 in ((0, 512), (512, 1024), (1024, 1152)):
                                    nc.tensor.matmul(
                                        dps[:, c0:c1], lslice, rhs_d[h][:, c0:c1],
                                        start=True, stop=True,
                                    )
                                if use_m:
                                    absd = apool.tile([P, KH * M], f32, tag="absd")
                                    nc.scalar.activation(absd[:], dps[:, 0:KH * M],
                                                         ACT.Abs)
                                    av = absd[:].rearrange("p (j b m) -> p j b m",
                                                           b=2, m=M)
                                    mxo = mpool.tile([P, NPAIR * M], f32, tag="mxo")
                                    nc.gpsimd.tensor_tensor(
                                        mxo[:].rearrange("p (j m) -> p j m", m=M),
                                        av[:, :, 0, :], av[:, :, 1, :], Alu.add)
                                    nc.vector.tensor_reduce(
                                        Dr3[:, t, h, :],
                                        mxo[:].rearrange("p (j m) -> p m j", m=M),
                                        axis=X, op=Alu.add,
                                    )
                                else:
                                    nc.vector.tensor_reduce(
                                        Dr3[:, t, h, :],
                                        dps[:, 0:KH * M].rearrange(
                                            "p (k m) -> p m k", m=M),
                                        axis=X, op=Alu.add,
                                        apply_absolute_value=True,
                                    )

                    nc.vector.tensor_tensor(d3(Dm[:]), Dr3[:, :, 0, :],
                                            Dr3[:, :, 1, :], Alu.add)
                sprep.release()

                # ---------------- phase 2 + 3 (PSUM scope for transposes) ------
                with tc.tile_pool(name="ptr", bufs=2, space="PSUM") as ptr:
                    dist = ft("dist")
                    nc.vector.tensor_tensor(d3(dist[:]), d3(Dm[:]), srep(4), Alu.mult)
                    mx3 = pool.tile([P, 3], f32)
                    nc.vector.tensor_reduce(mx3[:, 0:1], d3(dist[:]), axis=XY, op=Alu.max)
                    nc.vector.tensor_reduce(mx3[:, 1:2], d3(xyf[:]), axis=XY, op=Alu.max)
                    nc.vector.tensor_reduce(mx3[:, 2:3], d3(thf[:]), axis=XY, op=Alu.max)
                    mxT_ps = ptr.tile([3, P], f32, tag="mxT")
                    nc.tensor.transpose(mxT_ps[:], mx3[:], ident[:])
                    mxT = pool.tile([3, P], f32)
                    nc.scalar.activation(mxT[:], mxT_ps[:], ACT.Copy)
                    g3 = pool.tile([3, 1], f32)
                    nc.vector.tensor_reduce(g3[:], mxT[:], axis=X, op=Alu.max)
                    nc.sync.dma_start(out=scr_g[:], in_=g3[:])
                    gmx = pool.tile([P, 3], f32)
                    nc.sync.dma_start(
                        out=gmx[:],
                        in_=scr_g[:].flatten().unsqueeze(0).broadcast_to([P, 3]))
                    gmx2 = pool.tile([P, 3], f32)
                    nc.vector.tensor_scalar(gmx2[:], gmx[:], 1e-6, None, Alu.max)
                    nginv = pool.tile([P, 3], f32)
                    nc.vector.reciprocal(nginv[:], gmx2[:])
                    nc.vector.tensor_scalar(nginv[:], nginv[:], -1.0, None, Alu.mult)

                    xys = ft("xys")
                    nc.vector.tensor_scalar(xys[:], xyf[:], nginv[:, 1:2], 1.01,
                                            Alu.mult, Alu.add)
                    ths = ft("ths")
                    nc.vector.tensor_scalar(ths[:], thf[:], nginv[:, 2:3], 1.01,
                                            Alu.mult, Alu.add)
                    dsx = ft("dsx")
                    nc.vector.tensor_scalar(dsx[:], dist[:], nginv[:, 0:1], None,
                                            Alu.mult)
                    s3 = ft("s3")
                    nc.vector.scalar_tensor_tensor(s3[:], dsx[:], 1.01, xys[:],
                                                   Alu.add, Alu.mult)
                    nc.vector.tensor_tensor(s3[:], s3[:], ths[:], Alu.mult)
                    sq = ft("sq")
                    nc.scalar.activation(sq[:], s3[:], ACT.Square)
                    cost = ft("cost")
                    nc.vector.scalar_tensor_tensor(cost[:], sq[:], -3.0, cls[:],
                                                   Alu.mult, Alu.add)

                    # transposes; DTn first (only depends on Dm). The cost
                    # transpose-copy applies -cost*mask + BIG*(1-mask) via
                    # ACT per-partition scale/bias, so masked columns top out
                    # at +BIG and yield a -BIG threshold (no match-mask pass).
                    negMk = pool.tile([M, 1], f32)
                    nc.vector.tensor_scalar(negMk[:], Mk[:], -1.0, None, Alu.mult)
                    TnP = pool.tile([2 * M, n], f32)
                    c8src = TnP[0:M]
                    DTn = TnP[M:2 * M]
                    dv = Dm[:].rearrange("p (t m) -> p t m", m=M)
                    cv = cost[:].rearrange("p (t m) -> p t m", m=M)
                    for g in range(T // 4):
                        tpg = ptr.tile([M, 4 * P], f32, tag="tpD", name="tpD")
                        for j in range(4):
                            t = 4 * g + j
                            nc.tensor.transpose(tpg[:, j * P:(j + 1) * P],
                                                dv[:, t, :], ident[:])
                        nc.scalar.activation(DTn[:, g * 4 * P:(g + 1) * 4 * P],
                                             tpg[:], ACT.Copy, scale=-1.0)
                    for g in range(T // 4):
                        tpg = ptr.tile([M, 4 * P], f32, tag="tpC", name="tpC")
                        for j in range(4):
                            t = 4 * g + j
                            nc.tensor.transpose(tpg[:, j * P:(j + 1) * P],
                                                cv[:, t, :], ident[:])
                        nc.vector.tensor_scalar(
                            c8src[:, g * 4 * P:(g + 1) * 4 * P], tpg[:],
                            negMk[:], bigoff[:], Alu.mult, Alu.add)

                    d8 = pool.tile([M, 8], f32)
                    nc.vector.max(d8[:], DTn)
                    c8 = pool.tile([M, 8], f32)
                    nc.vector.max(c8[:], c8src)

                    num4 = pool.tile([M, 4], f32)
                    nc.vector.tensor_scalar(num4[:], d8[:, 0:4], a30[:], None, Alu.add)
                    den4 = pool.tile([M, 4], f32)
                    nc.vector.tensor_scalar(den4[:], d8[:, 0:4], -1.0, None, Alu.mult)
                    nc.vector.tensor_scalar(den4[:], den4[:], aeps[:], None, Alu.add)
                    rec4 = pool.tile([M, 4], f32)
                    nc.vector.reciprocal(rec4[:], den4[:])
                    iou4 = pool.tile([M, 4], f32)
                    nc.vector.tensor_tensor(iou4[:], num4[:], rec4[:], Alu.mult)
                    nc.vector.tensor_scalar(iou4[:], iou4[:], Mk[:], 0.0, Alu.mult, Alu.max)
                    S4 = pool.tile([M, 1], f32)
                    nc.vector.tensor_reduce(S4[:], iou4[:], axis=X, op=Alu.add)
                    ge2 = pool.tile([M, 1], f32)
                    nc.vector.tensor_scalar(ge2[:], S4[:], 2.0, None, Alu.is_ge)
                    ge3 = pool.tile([M, 1], f32)
                    nc.vector.tensor_scalar(ge3[:], S4[:], 3.0, None, Alu.is_ge)
                    ks = pool.tile([M, 1], f32)
                    nc.vector.tensor_scalar(ks[:], S4[:], 4.0, None, Alu.is_ge)
                    nc.vector.tensor_tensor(ks[:], ks[:], ge2[:], Alu.add)
                    nc.vector.tensor_tensor(ks[:], ks[:], ge3[:], Alu.add)
                    nc.vector.tensor_scalar(ks[:], ks[:], 1.0, None, Alu.add)

                    e4 = pool.tile([M, 4], f32)
                    nc.vector.tensor_scalar(e4[:], iota4[0:M, :], ks[:], None, Alu.is_equal)
                    tn4 = pool.tile([M, 4], f32)
                    nc.vector.tensor_tensor(tn4[:], c8[:, 0:4], e4[:], Alu.mult)
                    thn = pool.tile([M, 1], f32)
                    nc.vector.tensor_reduce(thn[:], tn4[:], axis=X, op=Alu.add)
                    nc.sync.dma_start(out=scr_thr[:], in_=thn[:])
                    ThrN = pool.tile([P, M], f32)
                    nc.sync.dma_start(
                        out=ThrN[:],
                        in_=scr_thr[:].flatten().unsqueeze(0).broadcast_to([P, M]))
                    Thr = pool.tile([P, M], f32)
                    nc.vector.tensor_scalar(Thr[:], ThrN[:], -1.0, None, Alu.mult)
                    thrb = Thr[:].unsqueeze(1).broadcast_to([P, T, M])

                    # phase 3
                    match = ft("match")
                    nc.vector.tensor_tensor(d3(match[:]), d3(cost[:]), thrb, Alu.is_le)
                    mgt = pool.tile([P, T], f32)
                    nc.vector.tensor_reduce(mgt[:], d3(match[:]), axis=X, op=Alu.add)

                    bmib = bmi[:].unsqueeze(1).broadcast_to([P, T, M])
                    pm1 = ft("pm1")
                    nc.gpsimd.tensor_tensor(d3(pm1[:]), d3(match[:]), bmib, Alu.mult)
                    i1r = pool.tile([P, T], f32)
                    nc.vector.tensor_reduce(i1r[:], d3(pm1[:]), axis=X, op=Alu.max)

                    nm1 = ft("nm1")
                    nc.vector.tensor_scalar(nm1[:], match[:], -1.0, 1.0, Alu.mult, Alu.add)
                    cm = ft("cm")
                    nc.vector.scalar_tensor_tensor(cm[:], nm1[:], BIG, cost[:],
                                                   Alu.mult, Alu.add)
                    mn2 = pool.tile([P, T], f32)
                    nc.vector.tensor_reduce(mn2[:], d3(cm[:]), axis=X, op=Alu.min)
                    mn2b = mn2[:].unsqueeze(2).broadcast_to([P, T, M])
                    eq2 = ft("eq2")
                    nc.vector.tensor_tensor(d3(eq2[:]), d3(cm[:]), mn2b, Alu.is_equal)
                    nc.vector.tensor_tensor(d3(eq2[:]), d3(eq2[:]), bmib, Alu.mult)
                    i2r = pool.tile([P, T], f32)
                    nc.vector.tensor_reduce(i2r[:], d3(eq2[:]), axis=X, op=Alu.max)

                    conf = pool.tile([P, T], f32)
                    nc.vector.tensor_scalar(conf[:], mgt[:], 1.0, None, Alu.is_gt)
                    asg = pool.tile([P, T], f32)
                    nc.vector.tensor_scalar(asg[:], mgt[:], 0.0, None, Alu.is_gt)
                    idx1 = pool.tile([P, T], f32)
                    nc.vector.tensor_scalar(idx1[:], i1r[:], -1.0, BIG2, Alu.mult, Alu.add)
                    idx2 = pool.tile([P, T], f32)
                    nc.vector.tensor_scalar(idx2[:], i2r[:], -1.0, BIG2, Alu.mult, Alu.add)
                    didx = pool.tile([P, T], f32)
                    nc.vector.tensor_tensor(didx[:], idx2[:], idx1[:], Alu.subtract)
                    nc.vector.tensor_tensor(didx[:], conf[:], didx[:], Alu.mult)
                    mt = pool.tile([P, T], f32)
                    nc.vector.tensor_tensor(mt[:], idx1[:], didx[:], Alu.add)
                    nc.vector.tensor_tensor(mt[:], mt[:], asg[:], Alu.mult)
                    nc.vector.tensor_tensor(mt[:], mt[:], asg[:], Alu.add)
                    nc.vector.tensor_scalar(mt[:], mt[:], -1.0, None, Alu.add)

                    asg_i = pool.tile([P, T], dt.int32)
                    nc.vector.tensor_copy(asg_i[:], asg[:])
                    mt_i = pool.tile([P, T], dt.int32)
                    nc.vector.tensor_copy(mt_i[:], mt[:])
                    nc.sync.dma_start(out=out_assigned[:].rearrange("(t p) -> p t", p=P),
                                      in_=asg_i[:])
                    nc.sync.dma_start(out=out_matched[:].rearrange("(t p) -> p t", p=P),
                                      in_=mt_i[:])
                ftp.release()

            for _rep in range(reps):
                _body()
    nc.compile()
    return nc


def _get_nc(n=N, reps=1):
    key = (n, reps)
    if key not in _CACHE:
        _CACHE[key] = _build_nc(n, reps)
    return _CACHE[key]


def kernel(preds, targets, masks, img_w=800, img_h=320):
    from concourse.bass_utils import run_bass_kernel_spmd

    nc = _get_nc(N)
    preds = np.ascontiguousarray(preds, dtype=np.float32)
    targets = np.ascontiguousarray(targets, dtype=np.float32)
    maskf = np.ascontiguousarray(masks, dtype=np.float32)
    in_maps = [
        {"preds": preds[b], "targets": targets[b], "maskf": maskf[b]}
        for b in range(B)
    ]
    res = run_bass_kernel_spmd(nc, in_maps, list(range(B))).results
    assigned = np.stack([res[b]["out_assigned"] for b in range(B)]).astype(bool)
    matched = np.stack([res[b]["out_matched"] for b in range(B)]).astype(np.int32)
    return assigned, matched

